# revision 1
# baseline (speedup 1.0000x reference)
"""DecoderRNN Trainium2 kernel.

Math (reference):
    emb = embed_table[captions]                      # (B, 31, E)
    inputs = concat([features[:,None,:], emb], 1)    # (B, T=32, E)
    xproj = inputs @ Wi + (bi + bh)                  # (B, T, H)
    h_t = tanh(xproj_t + h_{t-1} @ Wh)               # scan over T
    out = hs @ Wy + by                               # (B, T, V)

Distribution: vocab-parallel output projection across 8 cores (Wy/by sharded
by 1250 columns); the input GEMM and serial RNN are replicated full-batch on
every core (the RNN recurrence is inherently serial and weight-load bound,
so sharding it would not help). No collectives.

Design (measured ~88 us vs the 123 us v1 baseline):
  * Embedding gather + transpose on the host: kernel() uploads inputsT
    already transposed (E on partitions) and fp16. v1's device-side
    indirect-DMA gather + DRAM-scratch transpose serialized ~19 us before
    the first matmul and DMA-gated the input GEMM for ~40 us.
  * Few big input DMAs in need-order (issue costs ~650 ns on the Sync
    engine regardless of size; each InstDMACopy fans out over all 16 SDMA
    engines; the Q1 HWDGE ring completes them strictly in issue order).
  * RNN step = one identity matmul streaming xpT[t-1] into psum
    (start=True over all four [128,64] m-regions), 16 Wh matmuls
    accumulating on top, and a single fused tanh (ACT) evacuating psum to
    hsT. The serial chain is PE->ACT->PE only (~1.27 us/step); the vector
    engine is kept OFF the chain (its strict 8-deep FIFO would queue the
    critical op behind bulk projection work -- measured +18 us).
    NOTE: prefilling psum with a DVE copy instead of the identity matmul
    passes CoreSim but produces garbage on HW (PE accumulate over
    non-PE-written psum); keep the prefill on the PE.
  * Projection emitted after the RNN: the Tile static scheduler uses its
    independent matmuls to fill the PE idle inside the RNN chain.
  * Step t=1 is just tanh(xpT[0]) (h0 = 0): no matmuls.
  * ~10 garbage warmup matmuls bridge the initial DMA wait so the HAM
    clock gate is at full rate when the real GEMMs start.
  * The input GEMM's first block covers only timesteps 0-1 and its input
    columns land in a dedicated first DMA, so the RNN chain starts at
    ~13 us instead of ~19; the remaining blocks are emitted between RNN
    steps (producers before consumers) and hoisted into chain idle by the
    static scheduler. PSUM: 6 banks for GEMM chunks + 2 for the RNN.
  * fp16 output (halves the 10.2 MB/core store traffic; host upcasts),
    one store per 128-row tile.

On-chip layout keeps H (or E) on the partition axis everywhere:
    inputsT  [128, nb, k, c]   k = E/128 chunk, c = bt col within nb block
    xpT      [128, t, m*64+b]  m = H/128 chunk of the output
    hsT      [128, m, t*64+b]  t = 1..32 (slot 0 unused)
All matmul operands fp16 at peak rate (~0.42 ns/col; fp32 runs in slow
fp32-HIGH mode and disables fast weight load); accumulation fp32 in PSUM.
fp8 was measured and rejected: e4m3 hs/Wy gives 3.5% rel err vs the 2e-2
budget, and DoubleRow only pays off at free-dim >= 128 anyway.
"""

import sys

sys.path.insert(0, "/opt/trn_rl_repo")

from contextlib import ExitStack

import numpy as np

import concourse.bass as bass
import concourse.mybir as mybir
import concourse.tile as tile
from concourse import bacc
from concourse.bass import ts
from concourse.bass_utils import run_bass_kernel_spmd

B, T, E, H, V = 64, 32, 512, 512, 10000
NCORES = 8
VS = V // NCORES          # vocab shard per core
BT = B * T                # 2048 rows, t-major: row = t*64 + b
P = 128
KE = E // P               # 4 contraction chunks over E
KH = H // P               # 4 contraction chunks over H
MT = H // P               # 4 output chunks of H
NB = 4                    # bt blocks of 512 for the input GEMM
F32 = mybir.dt.float32
F16 = mybir.dt.float16

# projection N-chunks (psum bank holds 512 fp32 per partition)
VCHUNKS = [(0, 512), (512, 512), (1024, 226)]
assert sum(n for _, n in VCHUNKS) == VS


def build_program() -> bass.Bass:
    nc = bacc.Bacc()

    inp_head = nc.dram_tensor("inp_head", [P, KE, 128], F16, kind="ExternalInput")
    inp_rest = nc.dram_tensor("inp_rest", [P, KE, 384], F16, kind="ExternalInput")
    inp_tail = nc.dram_tensor("inp_tail", [P, NB - 1, KE, 512], F16, kind="ExternalInput")
    wi = nc.dram_tensor("wi", [P, KE, H], F16, kind="ExternalInput")
    wh = nc.dram_tensor("wh", [P, KH, MT, P], F16, kind="ExternalInput")
    bias = nc.dram_tensor("bias", [P, MT], F32, kind="ExternalInput")  # bi + bh
    wy = nc.dram_tensor("wy", [P, KH, VS], F16, kind="ExternalInput")
    by = nc.dram_tensor("by", [P, VS], F16, kind="ExternalInput")
    ident = nc.dram_tensor("ident", [P, P], F16, kind="ExternalInput")
    out = nc.dram_tensor("out", [BT, VS], F16, kind="ExternalOutput")

    with ExitStack() as ctx:
        tc = ctx.enter_context(tile.TileContext(nc))
        persist = ctx.enter_context(tc.tile_pool(name="persist", bufs=1))
        out_pool = ctx.enter_context(tc.tile_pool(name="outs", bufs=4))
        gemm_psum = ctx.enter_context(tc.tile_pool(name="gm_ps", bufs=6, space="PSUM"))
        rnn_psum = ctx.enter_context(tc.tile_pool(name="rn_ps", bufs=2, space="PSUM"))

        # ---- inputs: few big DMAs (issue costs ~650ns on Sync regardless of
        # size; one InstDMACopy already fans out over all 16 SDMA engines).
        # wi k=0 + the first bt block gate the first GEMM, so they go first.
        wi_sb = persist.tile([P, KE, H], F16, tag="wi")
        in_head = persist.tile([P, KE, 128], F16, tag="in_head")
        in_rest = persist.tile([P, KE, 384], F16, tag="in_rest")
        in_tail = persist.tile([P, NB - 1, KE, 512], F16, tag="in_tail")
        nc.sync.dma_start(out=wi_sb[:], in_=wi[:])
        # the first two timesteps' input columns are packed contiguously and
        # land first, so the RNN chain starts ~4us before the rest of the
        # first bt block arrives
        nc.sync.dma_start(out=in_head[:], in_=inp_head[:])

        ident_sb = persist.tile([P, P], F16, tag="ident")
        nc.sync.dma_start(out=ident_sb[:], in_=ident[:])

        bias_sb = persist.tile([P, MT], F32, tag="bias")
        nc.sync.dma_start(out=bias_sb[:], in_=bias[:])

        nc.sync.dma_start(out=in_rest[:], in_=inp_rest[:])

        wh_sb = persist.tile([P, KH, MT, P], F16, tag="wh")
        nc.sync.dma_start(out=wh_sb[:], in_=wh[:])

        # per-block tail DMAs: block 0 lands ~19us instead of ~24, so the
        # xproj block feeding RNN steps 10-17 is produced before step 10
        # consumes it (the single-DMA version stalled the chain ~1.5us at
        # each tail-block boundary)
        for nb in range(NB - 1):
            nc.sync.dma_start(out=in_tail[:, nb, :, :], in_=inp_tail[:, nb, :, :])

        wy_sb = persist.tile([P, KH, VS], F16, tag="wy")
        nc.sync.dma_start(out=wy_sb[:], in_=wy[:])

        by_rep = persist.tile([P, VS], F16, tag="by_rep")
        nc.sync.dma_start(out=by_rep[:], in_=by[:])

        # ---- HAM warmup: the PE clock-gate throttles to half rate when the
        # array is idle, and the real GEMMs can't start until the inputs land
        # (~13.5us). Keep the array busy on garbage during the DMA wait so
        # the first xproj matmuls run at full clock.
        warm = persist.tile([P, 512], F16, tag="warm")
        nc.vector.memset(warm[:], 0.0)
        wps = gemm_psum.tile([P, 512], F32, tag="gemm")
        for _ in range(10):
            nc.tensor.matmul(
                wps[:], lhsT=warm[:, 0:P], rhs=warm[:], start=True, stop=True,
                skip_group_check=True,
            )

        xpT = persist.tile([P, T, MT * B], F16, tag="xpT")
        hsT = persist.tile([P, MT, (T + 1) * B], F16, tag="hsT")

        # ---- xprojT = (inputs @ Wi).T + (bi + bh), in bt-blocks. The first
        # block covers only timesteps 0-1 (128 cols, ~1us) -- all the RNN
        # needs to start its chain; the rest of the blocks are emitted AFTER
        # the RNN loop so the static scheduler treats them as filler for the
        # chain's PE idle (like the projection) instead of head-blocking it.
        def xblock(rhs_tile, nb, t0, cn):
            for m in range(MT):
                ps = gemm_psum.tile([P, 512], F32, tag="gemm")
                for k in range(KE):
                    nc.tensor.matmul(
                        ps[:, :cn],
                        lhsT=wi_sb[:, k, ts(m, P)],
                        rhs=rhs_tile[:, k, :] if nb is None else rhs_tile[:, nb, k, :],
                        start=(k == 0),
                        stop=(k == KE - 1),
                    )
                # evacuate psum + per-partition bias on DVE, keeping ACT free
                # for the RNN tanhs (ACT is the RNN's serial-chain engine)
                nc.vector.tensor_scalar_add(
                    xpT[:, t0 : t0 + cn // B, ts(m, B)],
                    ps[:, :cn].rearrange("p (t b) -> p t b", b=B),
                    bias_sb[:, m : m + 1],
                )

        xblock(in_head, None, 0, 128)
        xblock(in_rest, None, 2, 384)

        # ---- RNN: hsT[t] = tanh(xpT[t-1] + Wh.T-chunks @ hsT[t-1])
        # h0 = 0, so step 1 is tanh(xpT[0]) with no matmuls.
        nc.scalar.activation(
            hsT[:, :, B : 2 * B],
            xpT[:, 0, :].rearrange("p (m b) -> p m b", b=B),
            mybir.ActivationFunctionType.Tanh,
        )
        # steps 2..32: an identity matmul streams xpT[t-1] into psum (one
        # [128,256] op covering all 4 m-regions, start=True), 16 Wh matmuls
        # accumulate on top, one fused tanh evacuates psum to hsT. The serial
        # chain stays on PE+ACT only; DVE never gates it. PE idle during tanh
        # is filled by xproj/projection matmuls.
        for t in range(2, T + 1):
            # emit each remaining xproj block right before the first RNN step
            # that consumes it (program order defines dataflow); the static
            # scheduler hoists its matmuls into earlier steps' PE idle
            if t in (9, 17, 25):
                nb = (t - 9) // 8
                xblock(in_tail, nb, 8 * (nb + 1), 512)
            rp = rnn_psum.tile([P, 512], F32, tag="rnn")
            nc.tensor.matmul(
                rp[:, 0 : MT * B],
                lhsT=ident_sb[:],
                rhs=xpT[:, t - 1, :],
                start=True,
                stop=False,
                skip_group_check=True,
            )
            for m in range(MT):
                for k in range(KH):
                    nc.tensor.matmul(
                        rp[:, ts(m, B)],
                        lhsT=wh_sb[:, k, m, :],
                        rhs=hsT[:, k, (t - 1) * B : t * B],
                        start=False,
                        stop=(k == KH - 1),
                        skip_group_check=True,
                    )
            nc.scalar.activation(
                hsT[:, :, t * B : (t + 1) * B],
                rp[:, 0 : MT * B].rearrange("p (m b) -> p m b", b=B),
                mybir.ActivationFunctionType.Tanh,
            )

        # ---- projection: out[bt_tile] = hs @ Wy + by  (emitted after the RNN;
        # independent per tile, so the static scheduler uses it to fill the
        # PE idle slots inside the RNN's serial chain)
        for i in range(BT // P):
            osb = out_pool.tile([P, VS], F16, tag="osb")
            for v0, vn in VCHUNKS:
                pp = gemm_psum.tile([P, 512], F32, tag="gemm")
                for k in range(KH):
                    nc.tensor.matmul(
                        pp[:, :vn],
                        lhsT=hsT[:, k, (2 * i + 1) * B : (2 * i + 1) * B + P],
                        rhs=wy_sb[:, k, v0 : v0 + vn],
                        start=(k == 0),
                        stop=(k == KH - 1),
                    )
                nc.vector.tensor_add(
                    osb[:, v0 : v0 + vn], pp[:, :vn], by_rep[:, v0 : v0 + vn]
                )
            # two stores per tile; by this point Sync has no input loads
            # left, and the shorter final transfer trims the kernel tail
            nc.sync.dma_start(out=out[ts(i, P), 0:640], in_=osb[:, 0:640])
            nc.sync.dma_start(out=out[ts(i, P), 640:VS], in_=osb[:, 640:VS])

    nc.compile()
    return nc


def make_in_maps(features, captions, embed_table, Wi, bi, Wh, bh, Wy, by):
    f32, f16 = np.float32, np.float16
    emb = np.asarray(embed_table, f32)[np.asarray(captions, np.int64)]  # (B,31,E)
    inputs = np.concatenate(
        [np.asarray(features, f32)[:, None, :], emb], axis=1
    )  # (B,T,E)
    inp_bt = np.ascontiguousarray(inputs.transpose(1, 0, 2).reshape(BT, E))
    # [p, nb, k, c] = inp_bt[nb*512 + c, k*128 + p]
    inpT = np.ascontiguousarray(
        inp_bt.reshape(NB, 512, KE, P).transpose(3, 0, 2, 1)
    ).astype(f16)
    inp_head = np.ascontiguousarray(inpT[:, 0, :, 0:128])
    inp_rest = np.ascontiguousarray(inpT[:, 0, :, 128:512])
    inp_tail = np.ascontiguousarray(inpT[:, 1:, :, :])
    wi_h = np.ascontiguousarray(
        np.asarray(Wi, f32).reshape(KE, P, H).transpose(1, 0, 2)
    ).astype(f16)
    wh_h = np.ascontiguousarray(
        np.asarray(Wh, f32).reshape(KH, P, MT, P).transpose(1, 0, 2, 3)
    ).astype(f16)
    bias_h = np.ascontiguousarray(
        (np.asarray(bi, f32) + np.asarray(bh, f32)).reshape(MT, P).T
    )
    wy_f = np.asarray(Wy, f32)
    by_f = np.asarray(by, f32)
    in_maps = []
    for c in range(NCORES):
        wy_h = np.ascontiguousarray(
            wy_f[:, c * VS : (c + 1) * VS].reshape(KH, P, VS).transpose(1, 0, 2)
        ).astype(f16)
        by_rep_h = np.ascontiguousarray(
            np.broadcast_to(by_f[c * VS : (c + 1) * VS], (P, VS))
        ).astype(f16)
        in_maps.append(
            {
                "inp_head": inp_head,
                "inp_rest": inp_rest,
                "inp_tail": inp_tail,
                "wi": wi_h,
                "wh": wh_h,
                "bias": bias_h,
                "wy": wy_h,
                "by": by_rep_h,
                "ident": np.eye(P, dtype=f16),
            }
        )
    return in_maps


def assemble(core_outs):
    full = np.concatenate(core_outs, axis=1)  # [BT, V] f16
    return np.ascontiguousarray(
        full.reshape(T, B, V).transpose(1, 0, 2)
    ).astype(np.float32)


def kernel(**inputs) -> np.ndarray:
    in_maps = make_in_maps(**inputs)
    nc = build_program()
    res = run_bass_kernel_spmd(nc, in_maps, core_ids=list(range(NCORES)))
    return assemble([r["out"] for r in res.results])

